# revision 1
# baseline (speedup 1.0000x reference)
"""KANvolution Trainium2 Bass kernel.

Math: the reference evaluates, per patch element x and per (f,c,ki,kj):
    K(x) = w_spline * sum_g basis_g(clip(x)) * cp_g  +  w_silu * silu(x)
with basis = normalized linear B-spline hats on a uniform 17-knot grid in
[-1,1].  The hat interpolant is piecewise-linear, so it can be rewritten
exactly as
    spline(x) = v0 + sum_{k=0..15} coef_k * relu(clip(x) - g_k)
(v0 folds into the bias).  That turns the whole module into a standard
3x3 valid conv over 17 feature maps of x (16 relus + silu), i.e. 9 taps
of matmuls with contraction K = 17*32 = 544 (padded to 5 k-tiles of 128,
with a constant-1 row carrying the bias).

Sharding: 8 cores = (batch b, output-row half).  Each core gets a
(34, 66, 32) input slab (2-row halo) and produces (32, 64, 64).

Device pipeline per core:
  DMA x (natural [spatial,c]) -> 18 PE transposes -> x^T [32c, 2304s]
  -> replicate x4 along partitions (SBUF->SBUF DMA) -> clip (DVE) ->
  4 relu k-tiles + silu k-tile (ACT, per-partition bias = -g) ->
  4 output chunks x 45 accumulating matmuls (float32r, N=512) ->
  PSUM -> copy -> PE transpose -> [128 m, 64 f] -> DMA out.
"""

import numpy as np
from contextlib import ExitStack

import concourse.bacc as bacc
import concourse.mybir as mybir
import concourse.tile as tile
from concourse.bass_utils import run_bass_kernel_spmd

# Problem constants (hardcoded per harness contract)
B, H, W, C, F = 4, 66, 66, 32, 64
KH = KW = 3
G = 16                      # spline intervals; G+1 = 17 knots
GRID_H = 2.0 / G            # 0.125
HO, WO = H - KH + 1, W - KW + 1          # 64, 64
N_CORES = 8
ROWS_PER_CORE = HO // 2                  # 32 output rows
IN_ROWS = ROWS_PER_CORE + KH - 1         # 34 input rows
SPAT = IN_ROWS * W                       # 2244 input spatial positions
SPAT_PAD = 18 * 128                      # 2304 (pad to 18 tiles of 128)
N_TAPS = KH * KW                         # 9
N_KTILES = 5                             # ceil(17*32/128) -> 4 relu tiles + silu tile
CHUNK_ROWS = 8                           # output rows per matmul chunk
N_CHUNKS = ROWS_PER_CORE // CHUNK_ROWS   # 4
NFREE = CHUNK_ROWS * WO                  # 512 moving-dim per matmul

_COMPILED = None  # cached (nc) program
import os
MM_DT = os.environ.get("KAN_MM_DT", "float32r")  # matmul operand dtype knob



def _build_weights(control_points, w_spline, w_silu, bias):
    """Host-side transform of the KAN params into conv-as-matmul weights.

    Returns w_host [128, 45*64] float32 and nothing else (bias folded in).
    Layout: w_host[p, (tap*5 + t)*64 + f] = W[tap][t][p, f] where row
    p = r*32 + c of k-tile t encodes feature g = 4t + r (t<4), and k-tile 4
    holds silu on rows 0..31, the bias row at partition 32 (tap 0 only).
    """
    cp = control_points.astype(np.float64)
    ws = w_spline.astype(np.float64)
    # hat interpolant values at the knots; the reference divides the hat
    # weights by (sum + 1e-8) with sum == 1, i.e. a uniform 1/(1+1e-8) scale
    v = ws[..., None] * cp / (1.0 + 1e-8)          # (F, C, 3, 3, 17)
    s = np.diff(v, axis=-1) / GRID_H               # (F, C, 3, 3, 16) slopes
    coef = s.copy()
    coef[..., 1:] = s[..., 1:] - s[..., :-1]       # slope deltas at knots 1..15
    v0 = v[..., 0]                                 # value at x = -1
    bias_eff = bias.astype(np.float64) + v0.sum(axis=(1, 2, 3))   # (F,)

    w_all = np.zeros((N_TAPS, N_KTILES, 128, F), dtype=np.float64)
    for i in range(KH):
        for j in range(KW):
            tap = i * KW + j
            for t in range(4):
                for r in range(4):
                    g = 4 * t + r
                    # rows r*32..r*32+32 = channels, feature g
                    w_all[tap, t, r * 32:(r + 1) * 32, :] = coef[:, :, i, j, g].T
            w_all[tap, 4, 0:32, :] = w_silu[:, :, i, j].astype(np.float64).T
    w_all[0, 4, 32, :] = bias_eff                  # constant-1 row, tap 0 only
    w_host = w_all.transpose(2, 0, 1, 3).reshape(128, N_TAPS * N_KTILES * F)
    import ml_dtypes
    npdt = np.float32 if MM_DT.startswith("float32") else ml_dtypes.bfloat16
    return np.ascontiguousarray(w_host.astype(npdt))


def _build_program():
    nc = bacc.Bacc("TRN2", target_bir_lowering=False, debug=False,
                   num_devices=N_CORES)
    f32 = mybir.dt.float32
    f32r = getattr(mybir.dt, MM_DT)

    xs_in = nc.declare_dram_parameter("xs", [SPAT_PAD, C], f32, isOutput=False)
    w_in = nc.declare_dram_parameter("w", [128, N_TAPS * N_KTILES * F], f32r,
                                     isOutput=False)
    gb_in = nc.declare_dram_parameter("gb", [128, 4], f32, isOutput=False)
    id_in = nc.declare_dram_parameter("ident", [128, 128], f32, isOutput=False)
    y_out = nc.declare_dram_parameter("y", [ROWS_PER_CORE * WO, F], f32,
                                      isOutput=True)

    with tile.TileContext(nc) as tc:
        with ExitStack() as ctx:
            sb = ctx.enter_context(tc.tile_pool(name="sb", bufs=1))
            ps_x = ctx.enter_context(tc.tile_pool(name="psx", bufs=2, space="PSUM"))
            ps_o = ctx.enter_context(tc.tile_pool(name="pso", bufs=3, space="PSUM"))
            ps_t = ctx.enter_context(tc.tile_pool(name="pst", bufs=2, space="PSUM"))
            ob = ctx.enter_context(tc.tile_pool(name="ob", bufs=2))

            # --- static inputs ---
            w_sb = sb.tile([128, N_TAPS * N_KTILES * F], f32r, tag="w")
            nc.sync.dma_start(w_sb[:], w_in[:])
            gb_sb = sb.tile([128, 4], f32, tag="gb")
            nc.sync.dma_start(gb_sb[:], gb_in[:])
            id_sb = sb.tile([128, 128], f32, tag="id")
            nc.sync.dma_start(id_sb[:], id_in[:])

            # --- load x natural layout: x_nat[p, (t,c)] = xs[t*128+p, c] ---
            x_nat = sb.tile([128, 18 * C], f32, tag="xnat")
            nc.sync.dma_start(
                x_nat[:].rearrange("p (t c) -> p t c", c=C),
                xs_in[:].rearrange("(t p) c -> p t c", p=128),
            )

            # --- transpose to x_rep[0:32] = x^T [c, spatial] ---
            x_rep = sb.tile([128, SPAT_PAD], f32, tag="xrep")
            for g in range(5):                       # groups of 4 transposes
                n_t = 4 if g < 4 else 2
                ps = ps_x.tile([32, 128 * n_t], f32, tag="psx")
                for u in range(n_t):
                    t = 4 * g + u
                    nc.tensor.transpose(
                        ps[:, 128 * u:128 * (u + 1)],
                        x_nat[:, C * t:C * (t + 1)],
                        id_sb[:],
                    )
                nc.scalar.copy(x_rep[0:32, 512 * g:512 * g + 128 * n_t], ps[:])

            # --- features, in 2 spatial halves so chunk-0 matmuls start early ---
            HALF = SPAT_PAD // 2
            xc = sb.tile([128, SPAT_PAD], f32, tag="xc")
            feats = [sb.tile([128, SPAT_PAD], f32r, name=f"feat{t}",
                             tag=f"feat{t}") for t in range(4)]
            f_silu = sb.tile([128, SPAT_PAD], f32r, tag="feat4")
            feats.append(f_silu)

            def make_features(h):
                cs = slice(HALF * h, HALF * (h + 1))
                # replicate x4 along partitions (SBUF->SBUF DMA)
                nc.sync.dma_start(x_rep[32:64, cs], x_rep[0:32, cs])
                nc.sync.dma_start(x_rep[64:96, cs], x_rep[0:32, cs])
                nc.sync.dma_start(x_rep[96:128, cs], x_rep[0:32, cs])
                nc.vector.tensor_scalar(xc[:, cs], x_rep[:, cs], 1.0, -1.0,
                                        mybir.AluOpType.min, mybir.AluOpType.max)
                for t in range(4):
                    if t < 2:   # ACT: relu(x + (-g))
                        nc.scalar.activation(feats[t][:, cs], xc[:, cs],
                                             mybir.ActivationFunctionType.Relu,
                                             bias=gb_sb[:, t:t + 1], scale=1.0)
                    else:       # DVE: (x - g) max 0, per-partition g
                        nc.vector.tensor_scalar(feats[t][:, cs], xc[:, cs],
                                                gb_sb[:, t:t + 1], 0.0,
                                                mybir.AluOpType.add,
                                                mybir.AluOpType.max)
                nc.scalar.activation(f_silu[:, cs], x_rep[:, cs],
                                     mybir.ActivationFunctionType.Silu)
                # constant-1 bias row: (x*0)+1 on DVE (memset can't write f32r)
                nc.vector.tensor_scalar(f_silu[32:33, cs], xc[32:33, cs],
                                        0.0, 1.0,
                                        mybir.AluOpType.mult,
                                        mybir.AluOpType.add)

            make_features(0)

            # --- conv as accumulating matmuls ---
            for q in range(N_CHUNKS):
                if q == 1:
                    make_features(1)
                po = ps_o.tile([F, NFREE], f32, tag="po")
                first = True
                for i in range(KH):
                    for j in range(KW):
                        tap = i * KW + j
                        base = (CHUNK_ROWS * q + i) * W
                        for t in range(N_KTILES):
                            rhs = (feats[t][:, base:base + CHUNK_ROWS * W]
                                   .rearrange("p (r w) -> p r w", w=W)
                                   [:, :, j:j + WO])
                            last = (tap == N_TAPS - 1) and (t == N_KTILES - 1)
                            col = (tap * N_KTILES + t) * F
                            nc.tensor.matmul(
                                po[:].rearrange("f (r w) -> f r w", w=WO),
                                w_sb[:, col:col + F],
                                rhs,
                                start=first, stop=last,
                            )
                            first = False

                # psum [64f, 512m] -> sbuf -> transpose -> [128m, 64f] -> DMA
                o_sb = ob.tile([F, NFREE], f32, tag="osb")
                nc.scalar.copy(o_sb[:], po[:])
                stage = ob.tile([128, 4 * F], f32, tag="stage")
                for u in range(4):
                    pt = ps_t.tile([128, F], f32, tag="pt")
                    nc.tensor.transpose(pt[:], o_sb[:, 128 * u:128 * (u + 1)],
                                        id_sb[0:F, 0:F])
                    nc.vector.tensor_copy(stage[:, F * u:F * (u + 1)], pt[:])
                nc.sync.dma_start(
                    y_out[NFREE * q:NFREE * (q + 1), :]
                        .rearrange("(tb p) f -> p tb f", p=128),
                    stage[:].rearrange("p (tb f) -> p tb f", f=F),
                )

    nc.compile()
    return nc


def _get_program():
    global _COMPILED
    if _COMPILED is None:
        _COMPILED = _build_program()
    return _COMPILED


def kernel(x, control_points, w_spline, w_silu, bias):
    x = np.asarray(x, dtype=np.float32)
    control_points = np.asarray(control_points, dtype=np.float32)
    w_spline = np.asarray(w_spline, dtype=np.float32)
    w_silu = np.asarray(w_silu, dtype=np.float32)
    bias = np.asarray(bias, dtype=np.float32)

    w_host = _build_weights(control_points, w_spline, w_silu, bias)
    grid = np.linspace(-1.0, 1.0, G + 1, dtype=np.float64)
    gb = np.zeros((128, 4), dtype=np.float32)
    for t in range(4):
        for p in range(128):
            gb[p, t] = -grid[4 * t + p // 32]
    ident = np.eye(128, dtype=np.float32)

    in_maps = []
    for core in range(N_CORES):
        b, half = divmod(core, 2)
        r0 = half * ROWS_PER_CORE
        xs = np.zeros((SPAT_PAD, C), dtype=np.float32)
        xs[:SPAT] = x[b, r0:r0 + IN_ROWS].reshape(SPAT, C)
        in_maps.append({"xs": xs, "w": w_host, "gb": gb, "ident": ident})

    nc = _get_program()
    res = run_bass_kernel_spmd(nc, in_maps, list(range(N_CORES)))

    out = np.empty((B, HO, WO, F), dtype=np.float32)
    for core in range(N_CORES):
        b, half = divmod(core, 2)
        r0 = half * ROWS_PER_CORE
        out[b, r0:r0 + ROWS_PER_CORE] = res.results[core]["y"].reshape(
            ROWS_PER_CORE, WO, F)
    return out



# revision 13
# speedup vs baseline: 1.7867x; 1.7867x over previous
"""KANvolution Trainium2 Bass kernel (v2: hat basis + bf16 + col-tiled PE).

Math: per patch element x and per (f,c,ki,kj):
    K(x) = w_spline * sum_g hat_g(clip(x)) * cp_g  +  w_silu * silu(x)
with hat_g the normalized linear B-spline (tent) basis on the 17-knot
grid in [-1,1] (hat sum == 1, so the reference's /(sum+1e-8) is a
constant 1/(1+1e-8) scale folded into the weights).

Instead of the relu-telescope decomposition (dense features, poorly
conditioned in bf16), we evaluate the tent basis directly:
    nhat_g(x) = min(|8*clip(x) - k_g| - 1, 0)   ( = -hat_g, k_g = 8*g_g )
and negate the spline weights host-side.  Only 2 of 17 hats are nonzero
per element and |v| ~ 1e-2, so bf16 matmul error is negligible.

Per tap (ki,kj) the contraction is 17 hats x 32 ch (+ silu x 32 + bias)
= 5 k-tiles of <=128.  9 taps x 5 k-tiles x 4 row-chunks = 180 matmuls
of [K<=128, N=64] x [K, M=512] per core, all bf16.

PE col-tiling: the F=64 output only fills half the 128-wide PE array,
so matmuls alternate between array column groups 0-63 / 64-127
(tile_position inferred from the PSUM slice base partition).  The two
groups stream concurrently through separate XBUSes -> ~2x throughput.
Each PSUM bank holds two independent accumulators (partitions 0-63 and
64-127); both halves are copied to SBUF, DMAed out, and summed on the
host.

Sharding: 8 cores = (batch b, output-row half).  Each core gets a
(34, 66, 32) input slab pre-transposed and pre-scaled (8x, bf16) on the
host and produces [128, 32*64] partial outputs (two 64-filter halves).
"""

import numpy as np
from contextlib import ExitStack

import concourse.bacc as bacc
import concourse.mybir as mybir
import concourse.tile as tile
from concourse.bass_utils import run_bass_kernel_spmd

# Problem constants (hardcoded per harness contract)
B, H, W, C, F = 4, 66, 66, 32, 64
KH = KW = 3
G = 16                                   # spline intervals; G+1 = 17 knots
HO, WO = H - KH + 1, W - KW + 1          # 64, 64
N_CORES = 8
ROWS_PER_CORE = HO // 2                  # 32 output rows
IN_ROWS = ROWS_PER_CORE + KH - 1         # 34 input rows
SPAT = IN_ROWS * W                       # 2244 input spatial positions
SPAT_PAD = 2304                          # pad to 18*128
N_TAPS = KH * KW                         # 9
N_KTILES = 5                             # 4 hat tiles (4x32 rows) + tail tile
K_TAIL = 65                              # tail tile rows: hat16(32)+silu(32)+bias(1)
CHUNK_ROWS = 8                           # output rows per matmul chunk
N_CHUNKS = ROWS_PER_CORE // CHUNK_ROWS   # 4
NFREE = CHUNK_ROWS * WO                  # 512 moving-dim per matmul
SLABS = [(0, 660), (660, 1188), (1188, 2304)]   # feature column slabs
N_WARMUP = 8                             # HAM warm-up matmuls

_COMPILED = None  # cached (nc) program


def _build_weights(control_points, w_spline, w_silu, bias):
    """Host-side transform of KAN params into the [128, 45*64] bf16 matrix.

    Column block (t*9 + tap)*64 .. +64 holds k-tile t of tap (ki,kj):
      t<4 : row r*32+c = knot g=4t+r, channel c, value -v[f,c,ki,kj,g]
      t=4 : rows 0-31 = knot 16 (negated), rows 32-63 = w_silu,
            row 64 = bias (tap 0 only).
    Hat weights are negated because the kernel computes -hat.
    """
    import ml_dtypes
    cp = control_points.astype(np.float64)
    ws = w_spline.astype(np.float64)
    v = ws[..., None] * cp / (1.0 + 1e-8)          # (F, C, 3, 3, 17)

    w_all = np.zeros((N_KTILES, N_TAPS, 128, F), dtype=np.float64)
    for i in range(KH):
        for j in range(KW):
            tap = i * KW + j
            for t in range(4):
                for r in range(4):
                    g = 4 * t + r
                    w_all[t, tap, r * 32:(r + 1) * 32, :] = -v[:, :, i, j, g].T
            w_all[4, tap, 0:32, :] = -v[:, :, i, j, 16].T
            w_all[4, tap, 32:64, :] = w_silu[:, :, i, j].astype(np.float64).T
    w_all[4, 0, 64, :] = bias.astype(np.float64)
    w_host = w_all.transpose(2, 0, 1, 3).reshape(128, N_KTILES * N_TAPS * F)
    return np.ascontiguousarray(w_host.astype(ml_dtypes.bfloat16))


def _build_program():
    nc = bacc.Bacc("TRN2", target_bir_lowering=False, debug=False,
                   num_devices=N_CORES)
    f32 = mybir.dt.float32
    bf16 = mybir.dt.bfloat16
    AF = mybir.ActivationFunctionType
    OP = mybir.AluOpType
    import os
    # CoreSim has no Silu; swap in Sigmoid for sim-only structure checks.
    AF_SILU = AF.Sigmoid if os.environ.get("KAN_SIM_SAFE") else AF.Silu

    x_in = nc.declare_dram_parameter("x8t", [32, SPAT_PAD], bf16, isOutput=False)
    w_in = nc.declare_dram_parameter("w", [128, N_KTILES * N_TAPS * F], bf16,
                                     isOutput=False)
    kv_in = nc.declare_dram_parameter("kv", [128, 8], f32, isOutput=False)
    ones_in = nc.declare_dram_parameter("ones", [1, SPAT_PAD], bf16,
                                        isOutput=False)
    y_out = nc.declare_dram_parameter("y", [128, N_CHUNKS * NFREE], f32,
                                      isOutput=True)

    with tile.TileContext(nc) as tc:
        with ExitStack() as ctx:
            sb = ctx.enter_context(tc.tile_pool(name="sb", bufs=1))
            ps = ctx.enter_context(tc.tile_pool(name="ps", bufs=3, space="PSUM"))
            ps_w = ctx.enter_context(tc.tile_pool(name="psw", bufs=1, space="PSUM"))
            ob = ctx.enter_context(tc.tile_pool(name="ob", bufs=2))

            # --- static inputs ---
            w_sb = sb.tile([128, N_KTILES * N_TAPS * F], bf16, tag="w")
            for t in range(N_KTILES):           # t-major blocks: first-needed first
                nc.sync.dma_start(w_sb[:, 576 * t:576 * (t + 1)],
                                  w_in[:, 576 * t:576 * (t + 1)])
            kv_sb = sb.tile([128, 8], f32, tag="kv")
            nc.sync.dma_start(kv_sb[:], kv_in[:])

            # x, replicated x4 along partitions (4 HBM reads)
            x_rep = sb.tile([128, SPAT_PAD], bf16, tag="xrep")
            for gg in range(4):
                nc.sync.dma_start(x_rep[32 * gg:32 * (gg + 1), :], x_in[:])

            # feature tiles
            xc8 = sb.tile([128, SPAT_PAD], bf16, tag="xc8")
            tb = [sb.tile([128, SPAT_PAD], bf16, name=f"tb{u}", tag=f"tb{u}")
                  for u in range(2)]
            nhat = [sb.tile([128, SPAT_PAD], bf16, name=f"nh{t}", tag=f"nh{t}")
                    for t in range(N_KTILES)]
            nc.sync.dma_start(nhat[4][64:65, :], ones_in[:])   # bias row = 1.0

            # warm the ACT table set (silu's set; relu/abs/copy are fillers)
            warm = sb.tile([1, 8], f32, tag="warm")
            nc.scalar.activation(warm[:], kv_sb[0:1, :], AF_SILU)

            # HAM warm-up: junk matmuls on the weight tile keep PE busy while
            # features are computed, so real matmuls run at full clock.
            pwarm = ps_w.tile([F, NFREE], f32, tag="pwarm")
            for u in range(N_WARMUP):
                nc.tensor.matmul(pwarm[:], w_sb[:, 0:F], w_sb[:, 0:NFREE],
                                 start=True, stop=True)

            def features(sl):
                a, b = SLABS[sl]
                cs = slice(a, b)
                # xc8 = clip(8x) to [-8, 8]
                nc.vector.tensor_scalar(xc8[:, cs], x_rep[:, cs], 8.0, -8.0,
                                        OP.min, OP.max)
                for t in range(4):
                    tbt = tb[t % 2]
                    # |xc8 - k| on ACT (per-partition bias = -k)
                    nc.scalar.activation(tbt[:, cs], xc8[:, cs], AF.Abs,
                                         bias=kv_sb[:, t:t + 1], scale=1.0)
                    nc.vector.tensor_scalar(nhat[t][:, cs], tbt[:, cs],
                                            1.0, 0.0, OP.subtract, OP.min)
                # tail tile: hat16 (rows 0-31), silu (rows 32-63)
                tbt = tb[0]
                nc.scalar.activation(tbt[0:32, cs], xc8[0:32, cs], AF.Abs,
                                     bias=kv_sb[0:32, 4:5], scale=1.0)
                nc.vector.tensor_scalar(nhat[4][0:32, cs], tbt[0:32, cs],
                                        1.0, 0.0, OP.subtract, OP.min)
                nc.scalar.activation(nhat[4][32:64, cs], x_rep[32:64, cs],
                                     AF_SILU, scale=0.125)

            def chunk(q):
                # two PSUM banks per chunk: group A (array cols 0-63) bank 0,
                # group B (cols 64-127) bank 1 — one accumulation group per
                # zero region.
                P = ps.tile([128, 2 * NFREE], f32, tag="po")
                order = [(t, tap) for t in range(N_KTILES)
                         for tap in range(N_TAPS)]
                grp_of = [(n + q) % 2 for n in range(len(order))]
                last = {}
                for n, g_ in enumerate(grp_of):
                    last[g_] = n
                started = [False, False]
                for n, (t, tap) in enumerate(order):
                    g_ = grp_of[n]
                    i, j = divmod(tap, KW)
                    base = (CHUNK_ROWS * q + i) * W
                    kk = 128 if t < 4 else K_TAIL
                    rhs = (nhat[t][0:kk, base:base + CHUNK_ROWS * W]
                           .rearrange("p (r w) -> p r w", w=W)
                           [:, :, j:j + WO])
                    col = (t * N_TAPS + tap) * F
                    nc.tensor.matmul(
                        P[F * g_:F * (g_ + 1), NFREE * g_:NFREE * (g_ + 1)]
                            .rearrange("f (r w) -> f r w", w=WO),
                        w_sb[0:kk, col:col + F],
                        rhs,
                        start=(not started[g_]), stop=(n == last[g_]),
                    )
                    started[g_] = True
                # PSUM -> SBUF (halves stay separate; host adds them)
                stage = ob.tile([128, NFREE], f32, tag="stage")
                nc.scalar.copy(stage[0:F, :], P[0:F, 0:NFREE])
                nc.vector.tensor_copy(stage[F:128, :], P[F:128, NFREE:2 * NFREE])
                nc.sync.dma_start(y_out[:, NFREE * q:NFREE * (q + 1)], stage[:])

            features(0)
            chunk(0)
            features(1)
            chunk(1)
            features(2)
            chunk(2)
            chunk(3)

    nc.compile()
    return nc


def _get_program():
    global _COMPILED
    if _COMPILED is None:
        _COMPILED = _build_program()
    return _COMPILED


def _make_in_maps(x, control_points, w_spline, w_silu, bias):
    import ml_dtypes
    bf = ml_dtypes.bfloat16
    w_host = _build_weights(control_points, w_spline, w_silu, bias)
    # ACT Abs bias: tb = Abs(xc8 + kv[:,t]) with kv = -(knot) = 8 - (4t + g)
    kv = np.zeros((128, 8), dtype=np.float32)
    for t in range(4):
        for p in range(128):
            kv[p, t] = 8.0 - (4 * t + p // 32)
    kv[:, 4] = -8.0                       # tail-tile (knot 16) Abs bias
    ones = np.ones((1, SPAT_PAD), dtype=bf)

    x8 = (np.asarray(x, dtype=np.float32) * 8.0).astype(bf)
    in_maps = []
    for core in range(N_CORES):
        b, half = divmod(core, 2)
        r0 = half * ROWS_PER_CORE
        xs = np.zeros((32, SPAT_PAD), dtype=bf)
        xs[:, :SPAT] = x8[b, r0:r0 + IN_ROWS].reshape(SPAT, C).T
        in_maps.append({"x8t": xs, "w": w_host, "kv": kv, "ones": ones})
    return in_maps


def kernel(x, control_points, w_spline, w_silu, bias):
    in_maps = _make_in_maps(x, control_points, w_spline, w_silu, bias)
    nc = _get_program()
    res = run_bass_kernel_spmd(nc, in_maps, list(range(N_CORES)))

    out = np.empty((B, HO, WO, F), dtype=np.float32)
    for core in range(N_CORES):
        b, half = divmod(core, 2)
        r0 = half * ROWS_PER_CORE
        y2 = res.results[core]["y"]                    # [128, 2048]
        y = y2[0:F] + y2[F:128]                        # [64, 2048]
        out[b, r0:r0 + ROWS_PER_CORE] = (
            y.reshape(F, ROWS_PER_CORE, WO).transpose(1, 2, 0))
    return out
